# revision 8
# baseline (speedup 1.0000x reference)
"""Squared-euclidean distance (VQ codebook) kernel for Trainium2.

dists[b,s,k] = ||x[b,s]||^2 - 2 x[b,s].C[k] + ||C[k]||^2

Strategy: data-parallel over the 8 NeuronCores — features [16,2048,512]
flatten to 32768 rows, 4096 rows per core; the [1024,512] codebook is
replicated.  The cross term is a [4096,512]@[512,1024] matmul per core
in bf16 (fp32 PSUM accumulate; bf16 streams at 1 cyc/row vs 4 for
fp32).  The features are pre-scaled by -2 on host (exact, power of
two), so PSUM holds -2*x.C directly.  ||x||^2 and ||C||^2 are computed
on host in fp64->fp32, shipped as one fp32 "aux" tensor ([128,32] x2
per-partition + [128,1024] c2 broadcast rows), and the epilogue is a
single VectorE scalar_tensor_tensor per tile:
    out = (psum + x2[row]) + c2[:]
keeping every instruction at <=2 semaphore waits (walrus limit).
"""

import numpy as np
import ml_dtypes

B, S, D, K = 16, 2048, 512, 1024
N_CORES = 8
ROWS = B * S                      # 32768
ROWS_PER_CORE = ROWS // N_CORES   # 4096
KT = D // 128                     # 4  contraction k-tiles
MT = ROWS_PER_CORE // 128         # 32 row tiles per core
G = 8                             # row groups of 512 rows (4 m-tiles each)
LM = MT // G                      # 4 m-tiles per group
NH = K // 512                     # 2 cluster halves of 512

_BF16 = ml_dtypes.bfloat16


def _split_multi_sync(nc):
    """Walrus codegen in this toolchain encodes at most ONE sync-wait (and one
    update) per 64-byte instruction ("Too many sync wait commands" otherwise).
    Tile's scheduler freely attaches several.  Hoist the extras onto standalone
    EventSemaphore instructions inserted just before (waits) / after (updates)
    on the same engine queue — semantically identical under in-order queues."""
    import concourse.mybir as mybir

    for bb in nc.main_func.blocks:
        insts = bb.instructions
        idx = 0
        while idx < len(insts):
            ins = insts[idx]
            si = ins.sync_info
            if si is None:
                idx += 1
                continue
            waits = list(si.on_wait or [])
            updates = list(si.on_update or [])
            if len(waits) <= 1 and len(updates) <= 1:
                idx += 1
                continue
            for j, w in enumerate(waits[:-1]):
                es = mybir.InstEventSemaphore(
                    name=f"{ins.name}_esw{j}", ins=[], outs=[]
                )
                es.engine = ins.engine
                es.sync_info = mybir.SyncInfo(on_wait=[w], on_update=[])
                insts.insert(idx, es)
                idx += 1
            for j, u in enumerate(updates[1:]):
                es = mybir.InstEventSemaphore(
                    name=f"{ins.name}_esu{j}", ins=[], outs=[]
                )
                es.engine = ins.engine
                es.sync_info = mybir.SyncInfo(on_wait=[], on_update=[u])
                insts.insert(idx + 1, es)
            ins.sync_info = mybir.SyncInfo(
                on_wait=waits[-1:], on_update=updates[:1]
            )
            idx += 1


def _build_bass():
    import concourse.bass as bass
    import concourse.mybir as mybir
    import concourse.tile as tile

    nc = bass.Bass(target_bir_lowering=False)

    # [g][p][k][r]: featT[g,p,k,r] = -2 * feat[g*512+r, k*128+p]
    featT = nc.dram_tensor(
        "featT", [G, 128, KT, 512], mybir.dt.bfloat16, kind="ExternalInput"
    )
    # [p][k][n]: ct[p,k,n] = C[n, k*128+p]
    ct = nc.dram_tensor("ct", [128, KT, K], mybir.dt.bfloat16, kind="ExternalInput")
    # aux[p, 0:MT] = x2 per-partition; aux[p, MT + n] = c2[n] (same all p)
    aux = nc.dram_tensor("aux", [128, MT + K], mybir.dt.float32, kind="ExternalInput")
    out = nc.dram_tensor(
        "out", [ROWS_PER_CORE, K], mybir.dt.float32, kind="ExternalOutput"
    )

    with tile.TileContext(nc) as tc:
        with (
            tc.tile_pool(name="singles", bufs=1) as singles,
            tc.tile_pool(name="feats", bufs=3) as feats,
            tc.tile_pool(name="stage", bufs=64) as stage_pool,
            tc.tile_pool(name="psum", bufs=4, space="PSUM") as psum_pool,
        ):
            ct_sb = singles.tile([128, KT, K], mybir.dt.bfloat16)
            nc.sync.dma_start(out=ct_sb, in_=ct[:, :, :])
            aux_sb = singles.tile([128, MT + K], mybir.dt.float32)
            nc.sync.dma_start(out=aux_sb, in_=aux[:, :])

            for g in range(G):
                feat_sb = feats.tile(
                    [128, KT, 512], mybir.dt.bfloat16, name=f"feat_{g}", tag="feat"
                )
                nc.sync.dma_start(out=feat_sb, in_=featT[g, :, :, :])
                for lm in range(LM):
                    mt = g * LM + lm
                    for nh in range(NH):
                        psum_t = psum_pool.tile(
                            [128, 512], mybir.dt.float32,
                            name=f"ps_{mt}_{nh}", tag="ps",
                        )
                        for k in range(KT):
                            nc.tensor.matmul(
                                psum_t,
                                feat_sb[:, k, lm * 128:(lm + 1) * 128],
                                ct_sb[:, k, nh * 512:(nh + 1) * 512],
                                start=(k == 0),
                                stop=(k == KT - 1),
                            )
                        st = stage_pool.tile(
                            [128, 512], mybir.dt.float32,
                            name=f"st_{mt}_{nh}", tag="st",
                        )
                        # st = (psum + x2[row]) + c2[:]
                        nc.vector.scalar_tensor_tensor(
                            out=st,
                            in0=psum_t,
                            scalar=aux_sb[:, mt:mt + 1],
                            in1=aux_sb[:, MT + nh * 512:MT + (nh + 1) * 512],
                            op0=mybir.AluOpType.add,
                            op1=mybir.AluOpType.add,
                        )
                        nc.sync.dma_start(
                            out=out[mt * 128:(mt + 1) * 128, nh * 512:(nh + 1) * 512],
                            in_=st,
                        )
    _split_multi_sync(nc)
    return nc


def _prep_inputs(features: np.ndarray, Ck: np.ndarray):
    """Host-side shard + layout prep. Returns list of per-core input dicts."""
    feat = np.ascontiguousarray(features.reshape(ROWS, D))
    C = np.ascontiguousarray(Ck.reshape(K, D))

    # replicated codebook tensors
    ct_host = np.ascontiguousarray(
        C.reshape(K, KT, 128).transpose(2, 1, 0)
    ).astype(_BF16)  # [p][k][n]
    c2_host = (C.astype(np.float64) ** 2).sum(-1).astype(np.float32)  # [K]

    in_maps = []
    for c in range(N_CORES):
        rows = feat[c * ROWS_PER_CORE:(c + 1) * ROWS_PER_CORE]
        featT_host = np.ascontiguousarray(
            (rows.reshape(G, 512, KT, 128) * np.float32(-2.0)).transpose(0, 3, 2, 1)
        ).astype(_BF16)  # [g][p][k][r], pre-scaled by -2
        x2_host = (rows.astype(np.float64) ** 2).sum(-1).astype(np.float32)
        aux_host = np.empty((128, MT + K), np.float32)
        aux_host[:, :MT] = x2_host.reshape(MT, 128).T
        aux_host[:, MT:] = c2_host[None, :]
        in_maps.append(
            {
                "featT": featT_host,
                "ct": ct_host,
                "aux": aux_host,
            }
        )
    return in_maps


_NC_CACHE = None


def _get_nc():
    global _NC_CACHE
    if _NC_CACHE is None:
        _NC_CACHE = _build_bass()
    return _NC_CACHE


def run(features: np.ndarray, Ck: np.ndarray, trace: bool = False):
    """Run on 8 cores; returns (full_output, BassKernelResults)."""
    from concourse.bass_utils import run_bass_kernel_spmd

    nc = _get_nc()
    in_maps = _prep_inputs(features, Ck)
    res = run_bass_kernel_spmd(
        nc, in_maps, core_ids=list(range(N_CORES)), trace=trace
    )
    parts = [r["out"] for r in res.results]
    full = np.concatenate(parts, axis=0).reshape(B, S, K).astype(np.float32)
    return full, res


def kernel(features: np.ndarray, Ck: np.ndarray) -> np.ndarray:
    full, _ = run(features, Ck, trace=False)
    return full
